# revision 18
# baseline (speedup 1.0000x reference)
"""BitNet Llama attention (B=2, S=2048, H=4096, 32 q-heads / 8 kv-heads, GQA),
distributed over 8 Trainium2 NeuronCores.

Sharding: token-sharded BitLinear QKV projections + activation quantization
(core c owns 512 consecutive global tokens), AllToAll to head-sharded
attention (core c = kv-head c + q-heads 4c..4c+3, full causal triangle —
identical instruction stream on every core, as SPMD requires), tiny
AllReduce/ReduceScatter(max) for the o-proj activation scales, quantize,
AllToAll back to token shards, BitLinear o_proj, host concat of row slices.

v2 restructuring for overlap:
- Phase A transposes batched (1 xbar-DMA per token tile instead of 32).
- K/V projections run first and their AllToAll fires early; Q projections
  are split into two 2-head halves, each followed by its own AllToAll, so
  the collectives ride the collective stream while the tensor engine keeps
  projecting (and the runtime's ~200us bootstrap barrier is absorbed).
- Attention runs per 2-head group as soon as that group's q lands.
- o is quantized and xbar-transposed on-chip before the return AllToAll,
  eliminating the old post-A2A transpose phase entirely.

BitLinear exactness: weights are ternarized on host and shipped as bf16
{-1,0,1}; activations are quantized on-chip to the int8 grid (magic-number
round-half-even) and stored as bf16 integers; bf16 x bf16 matmuls accumulate
exact integers in fp32 PSUM; per-token dequant scales are applied afterwards.
"""

import math
import os
import sys
from contextlib import ExitStack

import numpy as np
import ml_dtypes

for _p in ("/opt/trn_rl_repo", os.path.expanduser("~/.axon_site/_ro/trn_rl_repo")):
    if os.path.isdir(_p) and _p not in sys.path:
        sys.path.insert(0, _p)

import concourse.bass as bass
import concourse.mybir as mybir
import concourse.tile as tile
from concourse import bacc
from concourse.masks import make_identity

P = 128
H = 4096
DHEAD = 128
NH = 32
NKV = 8
NCORES = 8
MAGIC = 12582912.0  # 1.5 * 2**23: fp32 round-half-even via add/sub
LN2 = float(math.log(2.0))
INV_SQRT_D = float(np.float32(1.0) / np.float32(np.sqrt(np.float32(DHEAD))))
INV127 = float(np.float32(1.0) / np.float32(127.0))

F32 = mybir.dt.float32
BF16 = mybir.dt.bfloat16
MULT = mybir.AluOpType.mult
ADD = mybir.AluOpType.add
SUB = mybir.AluOpType.subtract
MAXOP = mybir.AluOpType.max


def build_program(S=2048, B=2):
    """One SPMD program; per-core behavior differs only through input data."""
    T_GLOB = B * S                      # global tokens
    T_OWN = T_GLOB // NCORES            # tokens owned per core
    NT = T_OWN // P                     # own token tiles (4 at S=2048)
    QTB = S // P                        # q tiles per batch (16)
    QT_ALL = B * QTB                    # global token tiles (32)
    HT = H // P                         # hidden tiles (32)
    GF = H // NCORES                    # q-features per head group (512)
    NVT = NKV * DHEAD // 512            # 512-wide V psum chunks (2)

    CH_K = P * T_OWN                    # k chunk elems (per dest rank)
    CH_V = NT * P * P                   # v chunk elems
    CH_KV = CH_K + CH_V
    CH_QH = 2 * P * T_OWN               # q chunk elems per half (2 head slots)
    CH_O = 4 * P * T_OWN                # o chunk elems (4 feature tiles)

    nc = bacc.Bacc(
        "TRN2", target_bir_lowering=False, debug=False, num_devices=NCORES
    )
    groups = [list(range(NCORES))]

    x_own = nc.dram_tensor("x_own", [T_OWN, H], F32, kind="ExternalInput")
    wqT = nc.dram_tensor("wqT", [H, H], BF16, kind="ExternalInput")
    wkT = nc.dram_tensor("wkT", [H, NKV * DHEAD], BF16, kind="ExternalInput")
    wvT = nc.dram_tensor("wvT", [H, NKV * DHEAD], BF16, kind="ExternalInput")
    woT = nc.dram_tensor("woT", [H, H], BF16, kind="ExternalInput")
    scal = nc.dram_tensor("scal", [P, 8], F32, kind="ExternalInput")
    cmaskT = nc.dram_tensor("cmaskT", [P, 4 * P], BF16, kind="ExternalInput")
    out_own = nc.dram_tensor("out_own", [T_OWN, H], F32, kind="ExternalOutput")

    with tile.TileContext(nc) as tc, ExitStack() as ctx:
        dram = ctx.enter_context(tc.tile_pool(name="dram", bufs=1, space="DRAM"))
        const = ctx.enter_context(tc.tile_pool(name="const", bufs=1))

        kv_in = dram.tile([NCORES, CH_KV], BF16, allow_tmpbuf=True)
        kv_out = dram.tile([NCORES, CH_KV], BF16, allow_tmpbuf=True)
        q_in = [
            dram.tile([NCORES, CH_QH], BF16, allow_tmpbuf=True, name=f"q_in{h}")
            for h in range(2)
        ]
        q_out = [
            dram.tile([NCORES, CH_QH], BF16, allow_tmpbuf=True, name=f"q_out{h}")
            for h in range(2)
        ]
        pamax_qm = dram.tile([T_GLOB], F32)         # token-major layout (for RS)
        amax_own_d = dram.tile([T_OWN], F32)
        oq_in = dram.tile([NCORES, CH_O], BF16, allow_tmpbuf=True)
        oq_out = dram.tile([NCORES, CH_O], BF16, allow_tmpbuf=True)

        ident = const.tile([P, P], BF16)
        make_identity(nc, ident)
        cmask_sb = const.tile([P, 4 * P], BF16)
        nc.sync.dma_start(cmask_sb[:], cmaskT[:, :])
        scal_sb = const.tile([P, 8], F32)
        nc.sync.dma_start(scal_sb[:], scal[:, :])

        pxq_cm = tc.tile_pool(name="pxq", bufs=1)
        pxq = pxq_cm.__enter__()
        xqT = pxq.tile([P, HT, T_OWN], BF16)           # quantized x, transposed
        dq_cols = pxq.tile([P, NT], F32)               # amax_clip/127 per own token

        # ---- Phase A: load x, quantize to int8 grid, transpose ----
        with tc.tile_pool(name="qwork", bufs=3) as qwork:
            for ti in range(NT):
                x_t = qwork.tile([P, H], F32, tag="x")
                nc.sync.dma_start(x_t[:], x_own[ti * P:(ti + 1) * P, :])
                amax = qwork.tile([P, 1], F32, tag="amax")
                nc.vector.tensor_reduce(
                    amax[:], x_t[:], mybir.AxisListType.X, MAXOP,
                    apply_absolute_value=True,
                )
                amax_c = qwork.tile([P, 1], F32, tag="amaxc")
                nc.vector.tensor_scalar(amax_c[:], amax[:], 1e-5, None, MAXOP)
                inv = qwork.tile([P, 1], F32, tag="inv")
                nc.vector.reciprocal(inv[:], amax_c[:])
                a_col = qwork.tile([P, 1], F32, tag="acol")
                nc.vector.tensor_scalar(a_col[:], inv[:], 127.0, None, MULT)
                nc.vector.tensor_scalar(
                    dq_cols[:, ti:ti + 1], amax_c[:], INV127, None, MULT
                )
                xr = qwork.tile([P, H], F32, tag="xr")
                nc.scalar.activation(
                    xr[:], x_t[:], mybir.ActivationFunctionType.Copy,
                    bias=MAGIC, scale=a_col[:],
                )
                xq = qwork.tile([P, H], BF16, tag="xq")
                nc.scalar.activation(
                    xq[:], xr[:], mybir.ActivationFunctionType.Copy, bias=-MAGIC
                )
                nc.scalar.dma_start_transpose(
                    xqT[:, :, ti * P:(ti + 1) * P], xq[:, :]
                )

        # ---- Phase A2: broadcast per-token dequant rows across partitions ----
        bcast_q = pxq.tile([P, T_OWN], F32)
        bcast_k = pxq.tile([P, T_OWN], F32)
        with tc.tile_pool(name="bwork", bufs=1) as bwork, \
             tc.tile_pool(name="psb", bufs=2, space="PSUM") as psb:
            dq_row = bwork.tile([1, T_OWN], F32)
            for ti in range(NT):
                nc.sync.dma_start(
                    dq_row[0:1, ti * P:(ti + 1) * P], dq_cols[:, ti:ti + 1]
                )
            ones_row = bwork.tile([1, P], F32)
            nc.vector.memset(ones_row[:], 1.0)
            srow_q = bwork.tile([1, T_OWN], F32)
            nc.vector.tensor_scalar(
                srow_q[:], dq_row[:], scal_sb[0:1, 0:1], INV_SQRT_D, MULT, MULT
            )
            srow_k = bwork.tile([1, T_OWN], F32)
            nc.vector.tensor_scalar(
                srow_k[:], dq_row[:], scal_sb[0:1, 1:2], None, MULT
            )
            for src, dst in ((srow_q, bcast_q), (srow_k, bcast_k)):
                ps = psb.tile([P, T_OWN], F32, tag="b")
                nc.tensor.matmul(ps[:], ones_row[:], src[:], start=True, stop=True)
                nc.vector.tensor_copy(dst[:], ps[:])

        # ---- Phase B: QKV projections (token-sharded) -> A2A chunks ----
        k_in_r = kv_in[:, 0:CH_K].rearrange("r (p t) -> r p t", p=P)
        v_in_r = kv_in[:, CH_K:CH_KV].rearrange("r (i p d) -> r i p d", i=NT, p=P)
        q_in_r = [
            q_in[h].rearrange("r (sl p t) -> r sl p t", sl=2, p=P) for h in range(2)
        ]
        wqT_r = wqT.rearrange("(hi p) o -> p hi o", p=P)
        wkT_r = wkT.rearrange("(hi p) o -> p hi o", p=P)
        wvT_r = wvT.rearrange("(hi p) o -> p hi o", p=P)

        with tc.tile_pool(name="wslab", bufs=4) as wslab, \
             tc.tile_pool(name="pevac", bufs=4) as pevac, \
             tc.tile_pool(name="psp", bufs=4, space="PSUM") as psp:
            # K projections (8 kv-head feature tiles) first
            for dj in range(NKV):
                wsl = wslab.tile([P, HT, P], BF16, tag="wq")
                nc.sync.dma_start(wsl[:], wkT_r[:, :, dj * P:(dj + 1) * P])
                ps = psp.tile([P, T_OWN], F32, tag="p")
                for hi in range(HT):
                    nc.tensor.matmul(
                        ps[:], wsl[:, hi, :], xqT[:, hi, :],
                        start=(hi == 0), stop=(hi == HT - 1),
                    )
                ev = pevac.tile([P, T_OWN], BF16, tag="e")
                nc.vector.tensor_tensor(ev[:], ps[:], bcast_k[:], MULT)
                nc.sync.dma_start(k_in_r[dj, :, :], ev[:])
            # V (natural layout)
            for vi in range(NVT):
                wsl = wslab.tile([P, HT, 512], BF16, tag="wv", bufs=2)
                nc.sync.dma_start(wsl[:], wvT_r[:, :, vi * 512:(vi + 1) * 512])
                for ti in range(NT):
                    ps = psp.tile([P, 512], F32, tag="pv")
                    for hi in range(HT):
                        nc.tensor.matmul(
                            ps[:], xqT[:, hi, ti * P:(ti + 1) * P], wsl[:, hi, :],
                            start=(hi == 0), stop=(hi == HT - 1),
                        )
                    sv = pevac.tile([P, 1], F32, tag="sv")
                    nc.vector.tensor_scalar(
                        sv[:], dq_cols[:, ti:ti + 1], scal_sb[:, 2:3], None, MULT
                    )
                    ev = pevac.tile([P, 512], BF16, tag="ev")
                    nc.scalar.mul(ev[:], ps[:], sv[:])
                    for sub in range(4):
                        nc.sync.dma_start(
                            v_in_r[vi * 4 + sub, ti, :, :],
                            ev[:, sub * P:(sub + 1) * P],
                        )
            # K/V AllToAll fires while Q projections continue below
            nc.gpsimd.collective_compute(
                "AllToAll", mybir.AluOpType.bypass, replica_groups=groups,
                ins=[kv_in[:, :].opt()], outs=[kv_out[:, :].opt()],
            )
            # Q projections, two halves of 2 head-slots each
            for half in range(2):
                djs = [d for d in range(HT) if d % 4 in (2 * half, 2 * half + 1)]
                for dj in djs:
                    wsl = wslab.tile([P, HT, P], BF16, tag="wq")
                    nc.sync.dma_start(wsl[:], wqT_r[:, :, dj * P:(dj + 1) * P])
                    ps = psp.tile([P, T_OWN], F32, tag="p")
                    for hi in range(HT):
                        nc.tensor.matmul(
                            ps[:], wsl[:, hi, :], xqT[:, hi, :],
                            start=(hi == 0), stop=(hi == HT - 1),
                        )
                    ev = pevac.tile([P, T_OWN], BF16, tag="e")
                    nc.vector.tensor_tensor(ev[:], ps[:], bcast_q[:], MULT)
                    nc.sync.dma_start(
                        q_in_r[half][dj // 4, (dj % 4) - 2 * half, :, :], ev[:]
                    )
                nc.gpsimd.collective_compute(
                    "AllToAll", mybir.AluOpType.bypass, replica_groups=groups,
                    ins=[q_in[half][:, :].opt()], outs=[q_out[half][:, :].opt()],
                )

        pxq_cm.__exit__(None, None, None)

        # ---- Phase D: assemble head-sharded attention operands ----
        amx_cm = tc.tile_pool(name="amx", bufs=1)
        amx = amx_cm.__enter__()
        amax_own_cols = amx.tile([P, NT], F32)
        pat_cm = tc.tile_pool(name="pat", bufs=1)
        pat = pat_cm.__enter__()
        qT_grp = pat.tile([P, 4, T_GLOB], BF16)
        kT_full = pat.tile([P, T_GLOB], BF16)
        v_full = pat.tile([P, QT_ALL, 132], BF16)
        nc.vector.memset(v_full[:], 1.0)  # column 128 = denominator ones
        nc.sync.dma_start(
            kT_full[:, :].rearrange("p (r t) -> p r t", r=NCORES),
            kv_out[:, 0:CH_K].rearrange("r (p t) -> p r t", p=P),
        )
        kv_out_v = kv_out[:, CH_K:CH_KV].rearrange("r (i p d) -> r p i d", i=NT, p=P)
        for r in range(NCORES):
            nc.sync.dma_start(
                v_full[:, r * NT:(r + 1) * NT, 0:P], kv_out_v[r]
            )
        for half in range(2):
            q_out_r = q_out[half].rearrange("r (sl p t) -> r p sl t", sl=2, p=P)
            for r in range(NCORES):
                nc.sync.dma_start(
                    qT_grp[:, 2 * half:2 * half + 2,
                           r * T_OWN:(r + 1) * T_OWN],
                    q_out_r[r],
                )

        # ---- Phase E: attention (full causal triangle, 2 heads per group) ----
        pos_cm = tc.tile_pool(name="pos", bufs=1)
        pos = pos_cm.__enter__()
        o_slice = pos.tile([P, QT_ALL, GF], F32)
        pamax_g = [pos.tile([P, QT_ALL], F32, name=f"pamax_g{g}") for g in range(2)]
        # AllReduce(min) of local 127/amax reciprocals == reciprocal of the
        # global amax max; each batch's exchange fires as soon as both head
        # groups finish that batch, hiding all but the last one.
        rc_b_sb = [pos.tile([P, QTB], F32, name=f"rc_b_sb{b}") for b in range(B)]
        rcol_b_pm = [
            dram.tile([P * QTB], F32, name=f"rcol_b_pm{b}") for b in range(B)
        ]
        rall_b_pm = [
            dram.tile([P * QTB], F32, name=f"rall_b_pm{b}") for b in range(B)
        ]
        rcol_qm = dram.tile([T_GLOB], F32)   # token-major local recips (for RS)
        with tc.tile_pool(name="att", bufs=4) as att, \
             tc.tile_pool(name="pss", bufs=4, space="PSUM") as pss, \
             tc.tile_pool(name="pso", bufs=4, space="PSUM") as pso:
            for g in range(2):
                for b in range(B):
                    for qb in range(QTB):
                        qt = b * QTB + qb
                        po = [
                            pso.tile([P, 132], F32, tag="o", name=f"po{g}_{_h}")
                            for _h in range(2)
                        ]
                        pt_all = att.tile([P, QTB, 2 * P], BF16, tag="pt", bufs=2)
                        for j in range(qb + 1):
                            kt = b * QTB + j
                            ps = pss.tile([P, 2 * P], F32, tag="s")
                            nc.tensor.matmul(
                                ps[:],
                                kT_full[:, kt * P:(kt + 1) * P],
                                qT_grp[:, 2 * g:2 * g + 2, qt * P:(qt + 1) * P],
                                start=True, stop=True,
                            )
                            nc.scalar.activation(
                                pt_all[:, j, :], ps[:],
                                mybir.ActivationFunctionType.Exp, scale=LN2,
                            )
                            if j == qb:
                                nc.vector.tensor_tensor(
                                    pt_all[:, j, :], pt_all[:, j, :],
                                    cmask_sb[:, 0:2 * P], MULT,
                                )
                            for hl in range(2):
                                nc.tensor.matmul(
                                    po[hl][:, 0:129],
                                    pt_all[:, j, hl * P:(hl + 1) * P],
                                    v_full[:, kt, 0:129],
                                    start=(j == 0), stop=(j == qb),
                                )
                        for hl in range(2):
                            den = att.tile([P, 1], F32, tag="den")
                            nc.vector.reciprocal(den[:], po[hl][:, 128:129])
                            nc.vector.tensor_scalar(
                                o_slice[:, qt, (2 * g + hl) * P:(2 * g + hl + 1) * P],
                                po[hl][:, 0:P], den[:], None, MULT,
                            )
                        nc.vector.tensor_reduce(
                            pamax_g[g][:, qt:qt + 1],
                            o_slice[:, qt, 2 * g * P:(2 * g + 2) * P],
                            mybir.AxisListType.X, MAXOP, apply_absolute_value=True,
                        )
                    if g == 1:
                        # batch b fully attended: local 127/amax recips and
                        # their AllReduce(min) — b0's hides under b1's rows
                        bs = slice(b * QTB, (b + 1) * QTB)
                        pm_b = pos.tile([P, QTB], F32, tag="pmb", name=f"pm_b{b}")
                        nc.vector.tensor_tensor(
                            pm_b[:], pamax_g[0][:, bs], pamax_g[1][:, bs], MAXOP
                        )
                        nc.vector.tensor_scalar(pm_b[:], pm_b[:], 1e-5, None, MAXOP)
                        nc.vector.reciprocal(rc_b_sb[b][:], pm_b[:])
                        nc.vector.tensor_scalar(
                            rc_b_sb[b][:], rc_b_sb[b][:], 127.0, None, MULT
                        )
                        nc.sync.dma_start(
                            rcol_b_pm[b][:].rearrange("(p q) -> p q", p=P),
                            rc_b_sb[b][:],
                        )
                        nc.sync.dma_start(
                            rcol_qm[b * QTB * P:(b + 1) * QTB * P].rearrange(
                                "(q p) -> p q", p=P
                            ),
                            rc_b_sb[b][:],
                        )
                        nc.gpsimd.collective_compute(
                            "AllReduce", mybir.AluOpType.min, replica_groups=groups,
                            ins=[rcol_b_pm[b][:].opt()],
                            outs=[rall_b_pm[b][:].opt()],
                        )
                        if b == B - 1:
                            nc.gpsimd.collective_compute(
                                "ReduceScatter", mybir.AluOpType.min,
                                replica_groups=groups,
                                ins=[rcol_qm[:].opt()], outs=[amax_own_d[:].opt()],
                            )

        # ---- Phase E2: quantize + transpose o per batch (b0's scales landed
        # mid-attention), return AllToAll ----
        nc.sync.dma_start(
            amax_own_cols[:, :],
            amax_own_d[:].rearrange("(t p) -> p t", p=P),
        )
        oT_cm = tc.tile_pool(name="oT", bufs=1)
        oTp = oT_cm.__enter__()
        oT_all = oTp.tile([P, 4, T_GLOB], BF16)
        with tc.tile_pool(name="oq", bufs=4) as oq:
            for b in range(B):
                a_colb = oq.tile([P, QTB], F32, tag="acolb", name=f"a_colb{b}")
                nc.sync.dma_start(
                    a_colb[:, :], rall_b_pm[b][:].rearrange("(p q) -> p q", p=P)
                )
                for qb2 in range(QTB):
                    qt = b * QTB + qb2
                    xr = oq.tile([P, GF], F32, tag="oxr")
                    nc.vector.tensor_scalar(
                        xr[:], o_slice[:, qt, :], a_colb[:, qb2:qb2 + 1],
                        MAGIC, MULT, ADD,
                    )
                    xq = oq.tile([P, GF], BF16, tag="oxq")
                    nc.scalar.activation(
                        xq[:], xr[:], mybir.ActivationFunctionType.Copy, bias=-MAGIC
                    )
                    eng = nc.sync if qt % 2 == 0 else nc.scalar
                    eng.dma_start_transpose(
                        oT_all[:, :, qt * P:(qt + 1) * P], xq[:, :]
                    )
                for s in range(b * (NCORES // B), (b + 1) * (NCORES // B)):
                    eng = nc.sync if s % 2 == 0 else nc.scalar
                    eng.dma_start(
                        oq_in[s:s + 1, :].rearrange(
                            "s (fi p t) -> s p fi t", fi=4, p=P
                        )[0],
                        oT_all[:, :, s * T_OWN:(s + 1) * T_OWN],
                    )
        nc.gpsimd.collective_compute(
            "AllToAll", mybir.AluOpType.bypass, replica_groups=groups,
            ins=[oq_in[:, :].opt()], outs=[oq_out[:, :].opt()],
        )
        oT_cm.__exit__(None, None, None)
        pos_cm.__exit__(None, None, None)
        pat_cm.__exit__(None, None, None)

        # ---- Phase G: o_proj (token-sharded, full output features) ----
        pxo_cm = tc.tile_pool(name="pxo", bufs=1)
        pxo = pxo_cm.__enter__()
        xoqT = pxo.tile([P, HT, T_OWN], BF16)
        with tc.tile_pool(name="gw", bufs=2) as gw, \
             tc.tile_pool(name="gev", bufs=3) as gev, \
             tc.tile_pool(name="psg", bufs=3, space="PSUM") as psg:
            woT_r = woT.rearrange("(hi p) o -> p hi o", p=P)
            # preload first o-proj weight slab before the A2A-dependent
            # assembles so it streams in during the collective tail
            wsl0 = gw.tile([P, HT, 512], BF16, tag="wo", name="wsl0")
            nc.sync.dma_start(wsl0[:], woT_r[:, :, 0:512])
            for r in range(NCORES):
                eng = nc.sync if r % 2 == 0 else nc.scalar
                eng.dma_start(
                    xoqT[:, 4 * r:4 * r + 4, :],
                    oq_out[r:r + 1, :].rearrange(
                        "s (fi p t) -> s p fi t", fi=4, p=P
                    )[0],
                )
            dqo_cols = gev.tile([P, NT], F32, tag="dqo")
            tmpc = gev.tile([P, NT], F32, tag="tc")
            nc.vector.reciprocal(tmpc[:], amax_own_cols[:])
            nc.vector.tensor_scalar(
                dqo_cols[:], tmpc[:], scal_sb[:, 3:4], None, MULT
            )
            for nj in range(H // 512):
                if nj == 0:
                    wsl = wsl0
                else:
                    wsl = gw.tile([P, HT, 512], BF16, tag="wo")
                    nc.sync.dma_start(wsl[:], woT_r[:, :, nj * 512:(nj + 1) * 512])
                for ti in range(NT):
                    ps = psg.tile([P, 512], F32, tag="g")
                    for hi in range(HT):
                        nc.tensor.matmul(
                            ps[:], xoqT[:, hi, ti * P:(ti + 1) * P], wsl[:, hi, :],
                            start=(hi == 0), stop=(hi == HT - 1),
                        )
                    ev = gev.tile([P, 512], F32, tag="ge")
                    nc.scalar.mul(ev[:], ps[:], dqo_cols[:, ti:ti + 1])
                    nc.sync.dma_start(
                        out_own[ti * P:(ti + 1) * P, nj * 512:(nj + 1) * 512], ev[:]
                    )

        pxo_cm.__exit__(None, None, None)
        amx_cm.__exit__(None, None, None)

    nc.compile()
    return nc


def _ternarize(W):
    ws = np.float32(max(np.mean(np.abs(W), dtype=np.float32), np.float32(1e-5)))
    t = np.clip(np.round(W / ws), -1.0, 1.0).astype(np.float32)
    return t, ws


def prepare_inputs(hidden_states, Wq, Wk, Wv, Wo, S=2048, B=2):
    bf16 = ml_dtypes.bfloat16
    T_GLOB = B * S
    T_OWN = T_GLOB // NCORES
    x = np.ascontiguousarray(
        np.asarray(hidden_states, dtype=np.float32).reshape(T_GLOB, H)
    )
    tq, wqs = _ternarize(np.asarray(Wq, dtype=np.float32))
    tk, wks = _ternarize(np.asarray(Wk, dtype=np.float32))
    tv, wvs = _ternarize(np.asarray(Wv, dtype=np.float32))
    to, wos = _ternarize(np.asarray(Wo, dtype=np.float32))
    wqT = np.ascontiguousarray(tq.T).astype(bf16)
    wkT = np.ascontiguousarray(tk.T).astype(bf16)
    wvT = np.ascontiguousarray(tv.T).astype(bf16)
    woT = np.ascontiguousarray(to.T).astype(bf16)
    scal = np.zeros((P, 8), np.float32)
    scal[:, 0] = wqs
    scal[:, 1] = wks
    scal[:, 2] = wvs
    scal[:, 3] = wos
    kk, qq = np.meshgrid(np.arange(P), np.arange(P), indexing="ij")
    cmaskT = np.tile((kk <= qq).astype(np.float32).astype(bf16), (1, 4))
    shared = dict(wqT=wqT, wkT=wkT, wvT=wvT, woT=woT, scal=scal, cmaskT=cmaskT)
    return [
        dict(x_own=np.ascontiguousarray(x[c * T_OWN:(c + 1) * T_OWN]), **shared)
        for c in range(NCORES)
    ]


_PROGRAM_CACHE = {}


def kernel(hidden_states, attention_mask, Wq, Wk, Wv, Wo):
    from concourse.bass_utils import run_bass_kernel_spmd

    B, S, _ = hidden_states.shape
    key = (B, S)
    if key not in _PROGRAM_CACHE:
        _PROGRAM_CACHE[key] = build_program(S=S, B=B)
    nc = _PROGRAM_CACHE[key]
    in_maps = prepare_inputs(hidden_states, Wq, Wk, Wv, Wo, S=S, B=B)
    res = run_bass_kernel_spmd(
        nc, in_maps, core_ids=list(range(NCORES)),
        trace=bool(int(os.environ.get("KERNEL_TRACE", "0"))),
    )
    out = np.concatenate([r["out_own"] for r in res.results], axis=0)
    kernel.last_results = res
    return np.ascontiguousarray(out.reshape(B, S, H)).astype(np.float32)
